# revision 19
# baseline (speedup 1.0000x reference)
"""MinLSTM layer on 8 Trainium2 NeuronCores.

Math (equivalent to the log-space reference, done in linear space):
    f_pre = x @ W_f.T + b_f ; i_pre = x @ W_i.T + b_i ; h_pre = x @ W_h.T + b_h
    sf = sigmoid(f_pre) ; si = sigmoid(i_pre)
    f = sf / (sf + si)                       # normalized forget gate
    i = 1 - f                                # = si / (sf + si)
    g = max(sigmoid(h_pre), h_pre + 0.5)     # == exp(log_g), exactly
    h_t = f_t * h_{t-1} + i_t * g_t,  h_0 = 1
The sf+si add and the f multiply run on the otherwise-idle GpSimd engine;
the DVE keeps reciprocal, gmax, mv and the scan (~112us busy vs the PE's
~168us -- enough slack that chain backlog drains before the kernel tail). (Softplus would fold the
normalization into ACT table lookups, but softplus and sigmoid never share
an ACT table, so the reloads would cost more than the DVE ops saved. A
GpSimd multiply was also tried: at ~2.5ns/elem it is 2x DVE cost and its
extra cross-engine hop lands on the pre-scan critical path.)

Precision: x and W are fed to the PE as fp16 (10 mantissa bits), fp32 PSUM
accumulate. Measured end-to-end max rel err vs the fp32 reference ~1.1e-3,
well inside the 2e-2 gate. fp16 beats fp32r here: the matmul streams at the
same 1 col/cycle, but LDWEIGHTS halves (FWL-eligible) so the back-to-back MM
gap is ~216ns vs ~229ns at N=512, and all input DMA bytes halve.

Sharding: 8 cores = batch(4) x hidden-halves(2). Core c handles batch b=c//2,
hidden slice [(c%2)*512, (c%2+1)*512). No cross-core communication; the scan
runs along T inside each core via the DVE TensorTensorScan instruction
(state = f*state - mv per step, mv = (f-1)*g = -i*g).

Schedule (per core):
- Warmup matmuls on a scratch tile (memset on the early-ready GpSimd engine)
  keep the PE busy from the end of the ~6us engine preamble through the slow
  first ~4us of the DMA queues' rate ramp, so the HAM clock gate reaches
  2.4 GHz right as the first real matmul's inputs land.
- All inputs ride the Sync hardware DMA ring in per-k (x[k], wf[k]) pairs
  (a Scalar-ring split was tried: the second ring's own slow ramp made
  time-to-first-data worse). A dummy sigmoid right after the warmups pulls
  the 1.3us ACT-table load into the warmup window.
- J0 (t 0..1024): f-gate k-outer spanning both 512-halves x 4 h-tiles = all
  8 PSUM banks, so each k-step consumes exactly one (x[k], wf[k]) DMA pair;
  W_i / W_h arrive as single 1MB descriptors during the f phase; the i and
  h phases run bank-outer k-inner, chasing the ACT engine's drains.
- J1+ chunks are h-tile-major; x arrives as one 3D DMA [128, 8k, nt] per
  512-half so the Sync queue's ~0.6us/descriptor issue cost stays off the
  critical path. Output stores also ride the Sync ring (the GpSimd ring is
  a software DGE -- much slower).
- With the DVE unsaturated, each unit's chain (~4us) hides inside the next
  unit's matmul phase, so only the final unit's sigmoid -> gmax -> mv ->
  scan -> store chain (~4us) trails the final matmul. (256-wide tail chunks
  and a gate-major final chunk were both tried and lose: the former stacks
  eight fixed-cost chains into the final window, the latter bunches all
  four h-tile chains after a 7us h-phase.)
"""

import sys

for _p in ("/opt/trn_rl_repo",):
    if _p not in sys.path:
        sys.path.append(_p)

import numpy as np

import concourse.bass as bass
import concourse.tile as tile
from concourse import bacc, mybir
from concourse.bass_utils import run_bass_kernel_spmd

B, T, DIN, DH = 4, 4096, 1024, 1024
N_CORES = 8
HSH = DH // 2          # 512 hidden channels per core
P = 128                # partitions
KT = DIN // P          # 8 contraction tiles
NT = 512               # matmul max t-chunk (free dim, one PSUM bank)
IT = HSH // P          # 4 h-tiles per core
CHUNKS = [(0, 1024), (1024, 1024), (2048, 1024), (3072, 512), (3584, 256), (3840, 256)]
NWARM = 8

MM_DT = mybir.dt.float16

_COMPILED = None


def _build():
    AF = mybir.ActivationFunctionType
    OP = mybir.AluOpType
    f32 = mybir.dt.float32

    nc = bacc.Bacc("TRN2", target_bir_lowering=False, debug=False)

    xT = nc.dram_tensor("xT", [DIN, T], MM_DT, kind="ExternalInput").ap()
    wd = {g: nc.dram_tensor(f"w{g}", [DIN, HSH], MM_DT, kind="ExternalInput").ap()
          for g in ("f", "i", "h")}
    # packed per-partition scalars: [b_f | b_i | b_h | b_h+0.5], each (128, IT)
    biases = nc.dram_tensor("biases", [P, 4 * IT], f32, kind="ExternalInput").ap()
    out = nc.dram_tensor("out", [HSH, T], f32, kind="ExternalOutput").ap()

    # DRAM views: (KT*P, n) -> [p, k, n]
    xT_v = xT.rearrange("(k p) t -> p k t", p=P)
    w_v = {g: w.rearrange("(k p) h -> p k h", p=P) for g, w in wd.items()}

    with tile.TileContext(nc) as tc:
        with (
            tc.tile_pool(name="wpool", bufs=1) as wpool,
            tc.tile_pool(name="bpool", bufs=1) as bpool,
            tc.tile_pool(name="xj0pool", bufs=1) as xj0pool,
            tc.tile_pool(name="xpool", bufs=4) as xpool,
            tc.tile_pool(name="psum", bufs=8, space="PSUM") as pspool,
            tc.tile_pool(name="work", bufs=4) as work,
            tc.tile_pool(name="hpool", bufs=6) as hpool,
        ):
            bias_t = bpool.tile([P, 4 * IT], f32, tag="bias")

            # weights: one [P, KT, HSH] resident tile per gate
            wt = {g: wpool.tile([P, KT, HSH], MM_DT, tag=f"w{g}", name=f"w{g}_t")
                  for g in ("f", "i", "h")}

            def bias_ap(kind, i):
                return bias_t[:, kind * IT + i:kind * IT + i + 1]

            hprev = [None] * IT
            hsls = [slice(i * P, (i + 1) * P) for i in range(IT)]

            def gmax(gt_sl, pst_sl, sg_sl, bias3, on_dve=True):
                """gt = max(h_pre + b_h + 0.5, sigmoid(h_pre)). DVE stt
                (GpSimd has no TensorTensor max; ACT has no max at all)."""
                nc.vector.scalar_tensor_tensor(
                    gt_sl, pst_sl, bias3, sg_sl, op0=OP.add, op1=OP.max)

            def fgate(uf, ui, ne):
                """f = sf/(sf+si) -> ui. The add runs on GpSimd (measured
                best: moving it to DVE anywhere, or the mul to GpSimd
                anywhere, both lose ~1us in 5-run medians)."""
                nc.gpsimd.tensor_add(ui[:], uf[:], ui[:])
                r = work.tile([P, ne], f32, tag="rcp", name="r_t")
                nc.vector.reciprocal_approx_fast(out=r[:], in_=ui[:])
                nc.vector.tensor_mul(ui[:], uf[:], r[:])

            def tail(i, fti, gt, J, t0, ne):
                """mv = (f-1)*g, scan, store. fti holds f."""
                nc.vector.scalar_tensor_tensor(
                    gt[:], fti[:], 1.0, gt[:], op0=OP.subtract, op1=OP.mult)
                hc = hpool.tile([P, ne], f32, tag="h", name=f"h{i}_t")
                init = 1.0 if J == 0 else hprev[i][:, -1:]
                nc.vector.tensor_tensor_scan(
                    hc[:], fti[:], gt[:], init, op0=OP.mult, op1=OP.subtract)
                hprev[i] = hc
                nc.sync.dma_start(
                    out=out[i * P:(i + 1) * P, t0:t0 + ne], in_=hc[:])

            # ---- warmup: PE busy from the end of the preamble so the HAM
            # clock gate ramps while the first input DMAs land.
            scratch = bpool.tile([P, NT], MM_DT, tag="scratch")
            nc.gpsimd.memset(scratch[:], 0.0)
            pswarm = pspool.tile([P, NT], f32, tag="ps", name="pswarm_t")
            for _ in range(NWARM):
                nc.tensor.matmul(pswarm[:], lhsT=scratch[:, :P], rhs=scratch[:],
                                 start=True, stop=True)

            # ---- J0: supply-aware first chunk ----
            t0, ne = CHUNKS[0]
            nhalf = ne // NT
            esls = [slice(h * NT, (h + 1) * NT) for h in range(nhalf)]

            # x on the Sync ring; bias + W on the Scalar ring, in parallel.
            # x[0] is split in half so the first matmul's dependency is 128KB.
            xj0 = [xj0pool.tile([P, ne], MM_DT, tag=f"xj0{k}", name=f"xj0{k}_t")
                   for k in range(KT)]
            for k in range(KT):
                if k == 0:
                    nc.sync.dma_start(out=xj0[0][:, :NT], in_=xT_v[:, 0, t0:t0 + NT])
                    nc.sync.dma_start(out=wt["f"][:, 0, :], in_=w_v["f"][:, 0, :])
                    nc.sync.dma_start(out=xj0[0][:, NT:], in_=xT_v[:, 0, t0 + NT:t0 + ne])
                    nc.sync.dma_start(out=bias_t[:], in_=biases[:])
                else:
                    nc.sync.dma_start(out=xj0[k][:], in_=xT_v[:, k, t0:t0 + ne])
                    nc.sync.dma_start(out=wt["f"][:, k, :], in_=w_v["f"][:, k, :])
            nc.sync.dma_start(out=wt["i"][:], in_=w_v["i"][:])
            nc.sync.dma_start(out=wt["h"][:], in_=w_v["h"][:])

            uf = [work.tile([P, ne], f32, tag="uf", name="uf_t") for _ in range(IT)]
            ui = [work.tile([P, ne], f32, tag="ui", name="ui_t") for _ in range(IT)]
            sg = [work.tile([P, ne], f32, tag="sg", name="sg_t") for _ in range(IT)]
            gt = [work.tile([P, ne], f32, tag="gt", name="gt_t") for _ in range(IT)]

            # f gate: k-outer across all 8 PSUM banks (2 halves x 4 h-tiles)
            # so each k-step consumes one (x[k], wf[k]) DMA pair.
            psf = [[pspool.tile([P, NT], f32, tag="ps", name="psf_t")
                    for _ in range(IT)] for _ in range(nhalf)]
            for k in range(KT):
                for half in range(nhalf):
                    for i in range(IT):
                        nc.tensor.matmul(
                            psf[half][i][:], lhsT=wt["f"][:, k, hsls[i]],
                            rhs=xj0[k][:, esls[half]],
                            start=(k == 0), stop=(k == KT - 1))
            for half in range(nhalf):
                for i in range(IT):
                    nc.scalar.activation(uf[i][:, esls[half]], psf[half][i][:],
                                         AF.Sigmoid, bias=bias_ap(0, i), scale=1.0)

            # i gate: bank-outer k-inner, chasing the freed f banks.
            for half in range(nhalf):
                for i in range(IT):
                    pst = pspool.tile([P, NT], f32, tag="ps", name="ps_t")
                    for k in range(KT):
                        nc.tensor.matmul(
                            pst[:], lhsT=wt["i"][:, k, hsls[i]],
                            rhs=xj0[k][:, esls[half]],
                            start=(k == 0), stop=(k == KT - 1))
                    nc.scalar.activation(ui[i][:, esls[half]], pst[:],
                                         AF.Sigmoid, bias=bias_ap(1, i), scale=1.0)
            for i in range(IT):
                fgate(uf[i], ui[i], ne)

            # h gate: bank-outer k-inner.
            for half in range(nhalf):
                for i in range(IT):
                    pst = pspool.tile([P, NT], f32, tag="ps", name="ps_t")
                    for k in range(KT):
                        nc.tensor.matmul(
                            pst[:], lhsT=wt["h"][:, k, hsls[i]],
                            rhs=xj0[k][:, esls[half]],
                            start=(k == 0), stop=(k == KT - 1))
                    nc.scalar.activation(sg[i][:, esls[half]], pst[:],
                                         AF.Sigmoid, bias=bias_ap(2, i), scale=1.0)
                    gmax(gt[i][:, esls[half]], pst[:], sg[i][:, esls[half]],
                         bias_ap(3, i))
            for i in range(IT):
                tail(i, ui[i], gt[i], 0, t0, ne)

            # ---- J1+: h-tile-major units ----
            for J, (t0, ne) in enumerate(CHUNKS[1:], start=1):
                halves = [(h0, min(NT, ne - h0)) for h0 in range(0, ne, NT)]
                xcs = []
                for h0, nt in halves:
                    xc = xpool.tile([P, KT, nt], MM_DT, tag="xh", name="xh_t")
                    nc.sync.dma_start(out=xc[:], in_=xT_v[:, :, t0 + h0:t0 + h0 + nt])
                    xcs.append(xc)
                for i in range(IT):
                    hsl = hsls[i]
                    ufu = work.tile([P, ne], f32, tag="uf", name="uf_t")
                    uiu = work.tile([P, ne], f32, tag="ui", name="ui_t")
                    sgu = work.tile([P, ne], f32, tag="sg", name="sg_t")
                    gtu = work.tile([P, ne], f32, tag="gt", name="gt_t")
                    for hi, (h0, nt) in enumerate(halves):
                        esl = slice(h0, h0 + nt)
                        for gate, dst, bk in (("f", ufu, 0), ("i", uiu, 1)):
                            pst = pspool.tile([P, NT], f32, tag="ps", name="ps_t")
                            for k in range(KT):
                                nc.tensor.matmul(
                                    pst[:, :nt], lhsT=wt[gate][:, k, hsl],
                                    rhs=xcs[hi][:, k, :],
                                    start=(k == 0), stop=(k == KT - 1))
                            nc.scalar.activation(dst[:, esl], pst[:, :nt],
                                                 AF.Sigmoid, bias=bias_ap(bk, i),
                                                 scale=1.0)
                    fgate(ufu, uiu, ne)
                    for hi, (h0, nt) in enumerate(halves):
                        esl = slice(h0, h0 + nt)
                        pst = pspool.tile([P, NT], f32, tag="ps", name="ps_t")
                        for k in range(KT):
                            nc.tensor.matmul(
                                pst[:, :nt], lhsT=wt["h"][:, k, hsl],
                                rhs=xcs[hi][:, k, :],
                                start=(k == 0), stop=(k == KT - 1))
                        nc.scalar.activation(sgu[:, esl], pst[:, :nt],
                                             AF.Sigmoid, bias=bias_ap(2, i),
                                             scale=1.0)
                        gmax(gtu[:, esl], pst[:, :nt], sgu[:, esl],
                             bias_ap(3, i))
                    tail(i, uiu, gtu, J, t0, ne)

    nc.compile()
    return nc


def _in_maps(x, W_f, b_f, W_i, b_i, W_h, b_h):
    x = np.asarray(x, np.float32)
    wT = {g: np.ascontiguousarray(np.asarray(w, np.float32).T).astype(np.float16)
          for g, w in (("f", W_f), ("i", W_i), ("h", W_h))}
    bs = {g: np.asarray(b, np.float32) for g, b in (("f", b_f), ("i", b_i), ("h", b_h))}

    maps = []
    for c in range(N_CORES):
        b, hh = divmod(c, 2)
        hsl = slice(hh * HSH, (hh + 1) * HSH)
        bias_pack = np.concatenate([
            bs["f"][hsl].reshape(IT, P).T,
            bs["i"][hsl].reshape(IT, P).T,
            bs["h"][hsl].reshape(IT, P).T,
            (bs["h"][hsl] + 0.5).reshape(IT, P).T,
        ], axis=1)
        maps.append({
            "xT": np.ascontiguousarray(x[b].T).astype(np.float16),
            "wf": np.ascontiguousarray(wT["f"][:, hsl]),
            "wi": np.ascontiguousarray(wT["i"][:, hsl]),
            "wh": np.ascontiguousarray(wT["h"][:, hsl]),
            "biases": np.ascontiguousarray(bias_pack, dtype=np.float32),
        })
    return maps


def kernel(x, W_f, b_f, W_i, b_i, W_h, b_h):
    global _COMPILED
    if _COMPILED is None:
        _COMPILED = _build()
    nc = _COMPILED

    res = run_bass_kernel_spmd(
        nc, _in_maps(x, W_f, b_f, W_i, b_i, W_h, b_h), list(range(N_CORES)))

    full = np.empty((B, T, DH), np.float32)
    for c in range(N_CORES):
        b, hh = divmod(c, 2)
        full[b, :, hh * HSH:(hh + 1) * HSH] = res.results[c]["out"].T
    return full


# revision 20
# speedup vs baseline: 1.0031x; 1.0031x over previous
"""MinLSTM layer on 8 Trainium2 NeuronCores.

Math (equivalent to the log-space reference, done in linear space):
    f_pre = x @ W_f.T + b_f ; i_pre = x @ W_i.T + b_i ; h_pre = x @ W_h.T + b_h
    sf = sigmoid(f_pre) ; si = sigmoid(i_pre)
    f = sf / (sf + si)                       # normalized forget gate
    i = 1 - f                                # = si / (sf + si)
    g = max(sigmoid(h_pre), h_pre + 0.5)     # == exp(log_g), exactly
    h_t = f_t * h_{t-1} + i_t * g_t,  h_0 = 1
The sf+si add runs on the GpSimd engine; the DVE keeps reciprocal,
multiply, gmax, mv and the scan. Measured dead ends: softplus-based
normalization (softplus and sigmoid never share an ACT table -> 1.3us
reloads); GpSimd multiply (~1.2us fixed cost + a cross-engine hop on the
pre-scan path); fp16 chain ops (the scan's serial carry is ~2.7ns/elem
regardless of dtype, so 16-bit does not speed the dominant DVE op).

Precision: x and W are fed to the PE as fp16 (10 mantissa bits), fp32 PSUM
accumulate. Measured end-to-end max rel err vs the fp32 reference ~1.1e-3,
well inside the 2e-2 gate. fp16 beats fp32r here: the matmul streams at the
same 1 col/cycle, but LDWEIGHTS halves (FWL-eligible) so the back-to-back MM
gap is ~216ns vs ~229ns at N=512, and all input DMA bytes halve.

Sharding: 8 cores = batch(4) x hidden-halves(2). Core c handles batch b=c//2,
hidden slice [(c%2)*512, (c%2+1)*512). No cross-core communication; the scan
runs along T inside each core via the DVE TensorTensorScan instruction
(state = f*state - mv per step, mv = (f-1)*g = -i*g).

Schedule (per core):
- Warmup matmuls on a scratch tile (memset on the early-ready GpSimd engine)
  keep the PE busy from the end of the ~6us engine preamble through the slow
  first ~4us of the DMA queues' rate ramp, so the HAM clock gate reaches
  2.4 GHz right as the first real matmul's inputs land.
- All inputs ride the Sync hardware DMA ring in per-k (x[k], wf[k]) pairs
  (a Scalar-ring split was tried: the second ring's own slow ramp made
  time-to-first-data worse; splitting wf[0] into smaller slices also lost --
  each extra ~0.6us descriptor slot delays every later k-step's supply).
- J0 (t 0..1024): f-gate k-outer spanning both 512-halves x 4 h-tiles = all
  8 PSUM banks, so each k-step consumes exactly one (x[k], wf[k]) DMA pair;
  W_i / W_h arrive as single 1MB descriptors during the f phase; the i and
  h phases run bank-outer k-inner, chasing the ACT engine's drains.
- J1+ chunks are h-tile-major; x arrives as one 3D DMA [128, 8k, nt] per
  512-half so the Sync queue's ~0.6us/descriptor issue cost stays off the
  critical path. Output stores also ride the Sync ring (the GpSimd ring is
  a software DGE -- much slower).
- The last two chunks are 256 wide to shorten the post-last-matmul serial
  chain. Tail variants measured within the +-1.5us run noise of each other
  (512-wide tail, gate-major final chunk, DVE-vs-GpSimd add placement); the
  shipped combination had the best 5-run median. Run-to-run exec variance
  is +-2us -- bench 5x before believing any schedule change.
"""

import sys

for _p in ("/opt/trn_rl_repo",):
    if _p not in sys.path:
        sys.path.append(_p)

import numpy as np

import concourse.bass as bass
import concourse.tile as tile
from concourse import bacc, mybir
from concourse.bass_utils import run_bass_kernel_spmd

B, T, DIN, DH = 4, 4096, 1024, 1024
N_CORES = 8
HSH = DH // 2          # 512 hidden channels per core
P = 128                # partitions
KT = DIN // P          # 8 contraction tiles
NT = 512               # matmul max t-chunk (free dim, one PSUM bank)
IT = HSH // P          # 4 h-tiles per core
CHUNKS = [(0, 1024), (1024, 1024), (2048, 1024), (3072, 512), (3584, 256), (3840, 256)]
NWARM = 8

MM_DT = mybir.dt.float16

_COMPILED = None


def _build():
    AF = mybir.ActivationFunctionType
    OP = mybir.AluOpType
    f32 = mybir.dt.float32

    nc = bacc.Bacc("TRN2", target_bir_lowering=False, debug=False)

    xT = nc.dram_tensor("xT", [DIN, T], MM_DT, kind="ExternalInput").ap()
    wd = {g: nc.dram_tensor(f"w{g}", [DIN, HSH], MM_DT, kind="ExternalInput").ap()
          for g in ("f", "i", "h")}
    # packed per-partition scalars: [b_f | b_i | b_h | b_h+0.5], each (128, IT)
    biases = nc.dram_tensor("biases", [P, 4 * IT], f32, kind="ExternalInput").ap()
    out = nc.dram_tensor("out", [HSH, T], f32, kind="ExternalOutput").ap()

    # DRAM views: (KT*P, n) -> [p, k, n]
    xT_v = xT.rearrange("(k p) t -> p k t", p=P)
    w_v = {g: w.rearrange("(k p) h -> p k h", p=P) for g, w in wd.items()}

    with tile.TileContext(nc) as tc:
        with (
            tc.tile_pool(name="wpool", bufs=1) as wpool,
            tc.tile_pool(name="bpool", bufs=1) as bpool,
            tc.tile_pool(name="xj0pool", bufs=1) as xj0pool,
            tc.tile_pool(name="xpool", bufs=4) as xpool,
            tc.tile_pool(name="psum", bufs=8, space="PSUM") as pspool,
            tc.tile_pool(name="work", bufs=4) as work,
            tc.tile_pool(name="hpool", bufs=6) as hpool,
        ):
            bias_t = bpool.tile([P, 4 * IT], f32, tag="bias")

            # weights: one [P, KT, HSH] resident tile per gate
            wt = {g: wpool.tile([P, KT, HSH], MM_DT, tag=f"w{g}", name=f"w{g}_t")
                  for g in ("f", "i", "h")}

            def bias_ap(kind, i):
                return bias_t[:, kind * IT + i:kind * IT + i + 1]

            hprev = [None] * IT
            hsls = [slice(i * P, (i + 1) * P) for i in range(IT)]

            def gmax(gt_sl, pst_sl, sg_sl, bias3, on_dve=True):
                """gt = max(h_pre + b_h + 0.5, sigmoid(h_pre)). DVE stt
                (GpSimd has no TensorTensor max; ACT has no max at all)."""
                nc.vector.scalar_tensor_tensor(
                    gt_sl, pst_sl, bias3, sg_sl, op0=OP.add, op1=OP.max)

            def fgate(uf, ui, ne):
                """f = sf/(sf+si) -> ui. The add runs on GpSimd (measured
                best: moving it to DVE anywhere, or the mul to GpSimd
                anywhere, both lose ~1us in 5-run medians)."""
                nc.gpsimd.tensor_add(ui[:], uf[:], ui[:])
                r = work.tile([P, ne], f32, tag="rcp", name="r_t")
                nc.vector.reciprocal_approx_fast(out=r[:], in_=ui[:])
                nc.vector.tensor_mul(ui[:], uf[:], r[:])

            def tail(i, fti, gt, J, t0, ne):
                """mv = (f-1)*g, scan, store. fti holds f."""
                nc.vector.scalar_tensor_tensor(
                    gt[:], fti[:], 1.0, gt[:], op0=OP.subtract, op1=OP.mult)
                hc = hpool.tile([P, ne], f32, tag="h", name=f"h{i}_t")
                init = 1.0 if J == 0 else hprev[i][:, -1:]
                nc.vector.tensor_tensor_scan(
                    hc[:], fti[:], gt[:], init, op0=OP.mult, op1=OP.subtract)
                hprev[i] = hc
                nc.sync.dma_start(
                    out=out[i * P:(i + 1) * P, t0:t0 + ne], in_=hc[:])

            # ---- warmup: PE busy from the end of the preamble so the HAM
            # clock gate ramps while the first input DMAs land.
            scratch = bpool.tile([P, NT], MM_DT, tag="scratch")
            nc.gpsimd.memset(scratch[:], 0.0)
            pswarm = pspool.tile([P, NT], f32, tag="ps", name="pswarm_t")
            for _ in range(NWARM):
                nc.tensor.matmul(pswarm[:], lhsT=scratch[:, :P], rhs=scratch[:],
                                 start=True, stop=True)

            # ---- J0: supply-aware first chunk ----
            t0, ne = CHUNKS[0]
            nhalf = ne // NT
            esls = [slice(h * NT, (h + 1) * NT) for h in range(nhalf)]

            # x on the Sync ring; bias + W on the Scalar ring, in parallel.
            # x[0] is split in half so the first matmul's dependency is 128KB.
            xj0 = [xj0pool.tile([P, ne], MM_DT, tag=f"xj0{k}", name=f"xj0{k}_t")
                   for k in range(KT)]
            for k in range(KT):
                if k == 0:
                    nc.sync.dma_start(out=xj0[0][:, :NT], in_=xT_v[:, 0, t0:t0 + NT])
                    nc.sync.dma_start(out=wt["f"][:, 0, :], in_=w_v["f"][:, 0, :])
                    nc.sync.dma_start(out=xj0[0][:, NT:], in_=xT_v[:, 0, t0 + NT:t0 + ne])
                    nc.sync.dma_start(out=bias_t[:], in_=biases[:])
                else:
                    nc.sync.dma_start(out=xj0[k][:], in_=xT_v[:, k, t0:t0 + ne])
                    nc.sync.dma_start(out=wt["f"][:, k, :], in_=w_v["f"][:, k, :])
            nc.sync.dma_start(out=wt["i"][:], in_=w_v["i"][:])
            nc.sync.dma_start(out=wt["h"][:], in_=w_v["h"][:])

            uf = [work.tile([P, ne], f32, tag="uf", name="uf_t") for _ in range(IT)]
            ui = [work.tile([P, ne], f32, tag="ui", name="ui_t") for _ in range(IT)]
            sg = [work.tile([P, ne], f32, tag="sg", name="sg_t") for _ in range(IT)]
            gt = [work.tile([P, ne], f32, tag="gt", name="gt_t") for _ in range(IT)]

            # f gate: k-outer across all 8 PSUM banks (2 halves x 4 h-tiles)
            # so each k-step consumes one (x[k], wf[k]) DMA pair.
            psf = [[pspool.tile([P, NT], f32, tag="ps", name="psf_t")
                    for _ in range(IT)] for _ in range(nhalf)]
            for k in range(KT):
                for half in range(nhalf):
                    for i in range(IT):
                        nc.tensor.matmul(
                            psf[half][i][:], lhsT=wt["f"][:, k, hsls[i]],
                            rhs=xj0[k][:, esls[half]],
                            start=(k == 0), stop=(k == KT - 1))
            for half in range(nhalf):
                for i in range(IT):
                    nc.scalar.activation(uf[i][:, esls[half]], psf[half][i][:],
                                         AF.Sigmoid, bias=bias_ap(0, i), scale=1.0)

            # i gate: bank-outer k-inner, chasing the freed f banks.
            for half in range(nhalf):
                for i in range(IT):
                    pst = pspool.tile([P, NT], f32, tag="ps", name="ps_t")
                    for k in range(KT):
                        nc.tensor.matmul(
                            pst[:], lhsT=wt["i"][:, k, hsls[i]],
                            rhs=xj0[k][:, esls[half]],
                            start=(k == 0), stop=(k == KT - 1))
                    nc.scalar.activation(ui[i][:, esls[half]], pst[:],
                                         AF.Sigmoid, bias=bias_ap(1, i), scale=1.0)
            for i in range(IT):
                fgate(uf[i], ui[i], ne)

            # h gate: bank-outer k-inner.
            for half in range(nhalf):
                for i in range(IT):
                    pst = pspool.tile([P, NT], f32, tag="ps", name="ps_t")
                    for k in range(KT):
                        nc.tensor.matmul(
                            pst[:], lhsT=wt["h"][:, k, hsls[i]],
                            rhs=xj0[k][:, esls[half]],
                            start=(k == 0), stop=(k == KT - 1))
                    nc.scalar.activation(sg[i][:, esls[half]], pst[:],
                                         AF.Sigmoid, bias=bias_ap(2, i), scale=1.0)
                    gmax(gt[i][:, esls[half]], pst[:], sg[i][:, esls[half]],
                         bias_ap(3, i))
            for i in range(IT):
                tail(i, ui[i], gt[i], 0, t0, ne)

            # ---- J1+: h-tile-major units ----
            for J, (t0, ne) in enumerate(CHUNKS[1:], start=1):
                halves = [(h0, min(NT, ne - h0)) for h0 in range(0, ne, NT)]
                xcs = []
                for h0, nt in halves:
                    xc = xpool.tile([P, KT, nt], MM_DT, tag="xh", name="xh_t")
                    nc.sync.dma_start(out=xc[:], in_=xT_v[:, :, t0 + h0:t0 + h0 + nt])
                    xcs.append(xc)
                for i in range(IT):
                    hsl = hsls[i]
                    ufu = work.tile([P, ne], f32, tag="uf", name="uf_t")
                    uiu = work.tile([P, ne], f32, tag="ui", name="ui_t")
                    sgu = work.tile([P, ne], f32, tag="sg", name="sg_t")
                    gtu = work.tile([P, ne], f32, tag="gt", name="gt_t")
                    for hi, (h0, nt) in enumerate(halves):
                        esl = slice(h0, h0 + nt)
                        for gate, dst, bk in (("f", ufu, 0), ("i", uiu, 1)):
                            pst = pspool.tile([P, NT], f32, tag="ps", name="ps_t")
                            for k in range(KT):
                                nc.tensor.matmul(
                                    pst[:, :nt], lhsT=wt[gate][:, k, hsl],
                                    rhs=xcs[hi][:, k, :],
                                    start=(k == 0), stop=(k == KT - 1))
                            nc.scalar.activation(dst[:, esl], pst[:, :nt],
                                                 AF.Sigmoid, bias=bias_ap(bk, i),
                                                 scale=1.0)
                    fgate(ufu, uiu, ne)
                    for hi, (h0, nt) in enumerate(halves):
                        esl = slice(h0, h0 + nt)
                        pst = pspool.tile([P, NT], f32, tag="ps", name="ps_t")
                        for k in range(KT):
                            nc.tensor.matmul(
                                pst[:, :nt], lhsT=wt["h"][:, k, hsl],
                                rhs=xcs[hi][:, k, :],
                                start=(k == 0), stop=(k == KT - 1))
                        nc.scalar.activation(sgu[:, esl], pst[:, :nt],
                                             AF.Sigmoid, bias=bias_ap(2, i),
                                             scale=1.0)
                        gmax(gtu[:, esl], pst[:, :nt], sgu[:, esl],
                             bias_ap(3, i))
                    tail(i, uiu, gtu, J, t0, ne)

    nc.compile()
    return nc


def _in_maps(x, W_f, b_f, W_i, b_i, W_h, b_h):
    x = np.asarray(x, np.float32)
    wT = {g: np.ascontiguousarray(np.asarray(w, np.float32).T).astype(np.float16)
          for g, w in (("f", W_f), ("i", W_i), ("h", W_h))}
    bs = {g: np.asarray(b, np.float32) for g, b in (("f", b_f), ("i", b_i), ("h", b_h))}

    maps = []
    for c in range(N_CORES):
        b, hh = divmod(c, 2)
        hsl = slice(hh * HSH, (hh + 1) * HSH)
        bias_pack = np.concatenate([
            bs["f"][hsl].reshape(IT, P).T,
            bs["i"][hsl].reshape(IT, P).T,
            bs["h"][hsl].reshape(IT, P).T,
            (bs["h"][hsl] + 0.5).reshape(IT, P).T,
        ], axis=1)
        maps.append({
            "xT": np.ascontiguousarray(x[b].T).astype(np.float16),
            "wf": np.ascontiguousarray(wT["f"][:, hsl]),
            "wi": np.ascontiguousarray(wT["i"][:, hsl]),
            "wh": np.ascontiguousarray(wT["h"][:, hsl]),
            "biases": np.ascontiguousarray(bias_pack, dtype=np.float32),
        })
    return maps


def kernel(x, W_f, b_f, W_i, b_i, W_h, b_h):
    global _COMPILED
    if _COMPILED is None:
        _COMPILED = _build()
    nc = _COMPILED

    res = run_bass_kernel_spmd(
        nc, _in_maps(x, W_f, b_f, W_i, b_i, W_h, b_h), list(range(N_CORES)))

    full = np.empty((B, T, DH), np.float32)
    for c in range(N_CORES):
        b, hh = divmod(c, 2)
        full[b, :, hh * HSH:(hh + 1) * HSH] = res.results[c]["out"].T
    return full


# revision 22
# speedup vs baseline: 1.0057x; 1.0026x over previous
"""MinLSTM layer on 8 Trainium2 NeuronCores.

Math (equivalent to the log-space reference, done in linear space):
    f_pre = x @ W_f.T + b_f ; i_pre = x @ W_i.T + b_i ; h_pre = x @ W_h.T + b_h
    sf = sigmoid(f_pre) ; si = sigmoid(i_pre)
    f = sf / (sf + si)                       # normalized forget gate
    i = 1 - f                                # = si / (sf + si)
    g = max(sigmoid(h_pre), h_pre + 0.5)     # == exp(log_g), exactly
    h_t = f_t * h_{t-1} + i_t * g_t,  h_0 = 1
The sf+si add runs on the GpSimd engine; the DVE keeps reciprocal,
multiply, gmax, mv and the scan. Measured dead ends: softplus-based
normalization (softplus and sigmoid never share an ACT table -> 1.3us
reloads); GpSimd multiply (~1.2us fixed cost + a cross-engine hop on the
pre-scan path); fp16 chain ops (the scan's serial carry is ~2.7ns/elem
regardless of dtype, so 16-bit does not speed the dominant DVE op).

Precision: x and W are fed to the PE as fp16 (10 mantissa bits), fp32 PSUM
accumulate. Measured end-to-end max rel err vs the fp32 reference ~1.1e-3,
well inside the 2e-2 gate. fp16 beats fp32r here: the matmul streams at the
same 1 col/cycle, but LDWEIGHTS halves (FWL-eligible) so the back-to-back MM
gap is ~216ns vs ~229ns at N=512, and all input DMA bytes halve.

Sharding: 8 cores = batch(4) x hidden-halves(2). Core c handles batch b=c//2,
hidden slice [(c%2)*512, (c%2+1)*512). No cross-core communication; the scan
runs along T inside each core via the DVE TensorTensorScan instruction
(state = f*state - mv per step, mv = (f-1)*g = -i*g).

Schedule (per core):
- Warmup matmuls on a scratch tile (memset on the early-ready GpSimd engine)
  keep the PE busy from the end of the ~6us engine preamble through the slow
  first ~4us of the DMA queues' rate ramp, so the HAM clock gate reaches
  2.4 GHz right as the first real matmul's inputs land.
- All inputs ride the Sync hardware DMA ring in per-k (x[k], wf[k]) pairs
  (a Scalar-ring split was tried: the second ring's own slow ramp made
  time-to-first-data worse; splitting wf[0] into smaller slices also lost --
  each extra ~0.6us descriptor slot delays every later k-step's supply).
- J0 (t 0..1024): f-gate k-outer spanning both 512-halves x 4 h-tiles = all
  8 PSUM banks, so each k-step consumes exactly one (x[k], wf[k]) DMA pair;
  W_i / W_h arrive as single 1MB descriptors during the f phase; the i and
  h phases run bank-outer k-inner, chasing the ACT engine's drains.
- J1+ chunks are h-tile-major; x arrives as one 3D DMA [128, 8k, nt] per
  512-half so the Sync queue's ~0.6us/descriptor issue cost stays off the
  critical path. Output stores also ride the Sync ring (the GpSimd ring is
  a software DGE -- much slower).
- The last two chunks are 256 wide to shorten the post-last-matmul serial
  chain. Tail variants measured within the +-1.5us run noise of each other
  (512-wide tail, gate-major final chunk, DVE-vs-GpSimd add placement); the
  shipped combination had the best 5-run median. Run-to-run exec variance
  is +-2us -- bench 5x before believing any schedule change.
"""

import sys

for _p in ("/opt/trn_rl_repo",):
    if _p not in sys.path:
        sys.path.append(_p)

import numpy as np

import concourse.bass as bass
import concourse.tile as tile
from concourse import bacc, mybir
from concourse.bass_utils import run_bass_kernel_spmd

B, T, DIN, DH = 4, 4096, 1024, 1024
N_CORES = 8
HSH = DH // 2          # 512 hidden channels per core
P = 128                # partitions
KT = DIN // P          # 8 contraction tiles
NT = 512               # matmul max t-chunk (free dim, one PSUM bank)
IT = HSH // P          # 4 h-tiles per core
CHUNKS = [(0, 1024), (1024, 1024), (2048, 1024), (3072, 512), (3584, 256), (3840, 256)]
NWARM = 8

MM_DT = mybir.dt.float16

_COMPILED = None


def _build():
    AF = mybir.ActivationFunctionType
    OP = mybir.AluOpType
    f32 = mybir.dt.float32

    nc = bacc.Bacc("TRN2", target_bir_lowering=False, debug=False)

    xT = nc.dram_tensor("xT", [DIN, T], MM_DT, kind="ExternalInput").ap()
    wd = {g: nc.dram_tensor(f"w{g}", [DIN, HSH], MM_DT, kind="ExternalInput").ap()
          for g in ("f", "i", "h")}
    # packed per-partition scalars: [b_f | b_i | b_h | b_h+0.5], each (128, IT)
    biases = nc.dram_tensor("biases", [P, 4 * IT], f32, kind="ExternalInput").ap()
    out = nc.dram_tensor("out", [HSH, T], f32, kind="ExternalOutput").ap()

    # DRAM views: (KT*P, n) -> [p, k, n]
    xT_v = xT.rearrange("(k p) t -> p k t", p=P)
    w_v = {g: w.rearrange("(k p) h -> p k h", p=P) for g, w in wd.items()}

    with tile.TileContext(nc) as tc:
        with (
            tc.tile_pool(name="wpool", bufs=1) as wpool,
            tc.tile_pool(name="bpool", bufs=1) as bpool,
            tc.tile_pool(name="xj0pool", bufs=1) as xj0pool,
            tc.tile_pool(name="xpool", bufs=4) as xpool,
            tc.tile_pool(name="psum", bufs=8, space="PSUM") as pspool,
            tc.tile_pool(name="work", bufs=4) as work,
            tc.tile_pool(name="hpool", bufs=6) as hpool,
        ):
            bias_t = bpool.tile([P, 4 * IT], f32, tag="bias")

            # weights: one [P, KT, HSH] resident tile per gate
            wt = {g: wpool.tile([P, KT, HSH], MM_DT, tag=f"w{g}", name=f"w{g}_t")
                  for g in ("f", "i", "h")}

            def bias_ap(kind, i):
                return bias_t[:, kind * IT + i:kind * IT + i + 1]

            hprev = [None] * IT
            hsls = [slice(i * P, (i + 1) * P) for i in range(IT)]

            def gmax(gt_sl, pst_sl, sg_sl, bias3, on_dve=True):
                """gt = max(h_pre + b_h + 0.5, sigmoid(h_pre)). DVE stt
                (GpSimd has no TensorTensor max; ACT has no max at all)."""
                nc.vector.scalar_tensor_tensor(
                    gt_sl, pst_sl, bias3, sg_sl, op0=OP.add, op1=OP.max)

            def fgate(uf, ui, ne):
                """f = sf/(sf+si) -> ui. The add runs on GpSimd (measured
                best: moving it to DVE anywhere, or the mul to GpSimd
                anywhere, both lose ~1us in 5-run medians)."""
                nc.gpsimd.tensor_add(ui[:], uf[:], ui[:])
                r = work.tile([P, ne], f32, tag="rcp", name="r_t")
                nc.vector.reciprocal_approx_fast(out=r[:], in_=ui[:])
                nc.vector.tensor_mul(ui[:], uf[:], r[:])

            def tail(i, fti, gt, J, t0, ne):
                """mv = (f-1)*g, scan, store. fti holds f."""
                nc.vector.scalar_tensor_tensor(
                    gt[:], fti[:], 1.0, gt[:], op0=OP.subtract, op1=OP.mult)
                hc = hpool.tile([P, ne], f32, tag="h", name=f"h{i}_t")
                init = 1.0 if J == 0 else hprev[i][:, -1:]
                nc.vector.tensor_tensor_scan(
                    hc[:], fti[:], gt[:], init, op0=OP.mult, op1=OP.subtract)
                hprev[i] = hc
                nc.sync.dma_start(
                    out=out[i * P:(i + 1) * P, t0:t0 + ne], in_=hc[:])

            # ---- warmup: PE busy from the end of the preamble so the HAM
            # clock gate ramps while the first input DMAs land.
            scratch = bpool.tile([P, NT], MM_DT, tag="scratch")
            nc.gpsimd.memset(scratch[:], 0.0)
            pswarm = pspool.tile([P, NT], f32, tag="ps", name="pswarm_t")
            for _ in range(NWARM):
                nc.tensor.matmul(pswarm[:], lhsT=scratch[:, :P], rhs=scratch[:],
                                 start=True, stop=True)

            # ---- J0: supply-aware first chunk ----
            t0, ne = CHUNKS[0]
            nhalf = ne // NT
            esls = [slice(h * NT, (h + 1) * NT) for h in range(nhalf)]

            # x on the Sync ring; bias + W on the Scalar ring, in parallel.
            # x[0] is split in half so the first matmul's dependency is 128KB.
            xj0 = [xj0pool.tile([P, ne], MM_DT, tag=f"xj0{k}", name=f"xj0{k}_t")
                   for k in range(KT)]
            for k in range(KT):
                if k == 0:
                    nc.sync.dma_start(out=xj0[0][:, :NT], in_=xT_v[:, 0, t0:t0 + NT])
                    nc.sync.dma_start(out=wt["f"][:, 0, :], in_=w_v["f"][:, 0, :])
                    nc.sync.dma_start(out=xj0[0][:, NT:], in_=xT_v[:, 0, t0 + NT:t0 + ne])
                    nc.sync.dma_start(out=bias_t[:], in_=biases[:])
                else:
                    nc.sync.dma_start(out=xj0[k][:], in_=xT_v[:, k, t0:t0 + ne])
                    nc.sync.dma_start(out=wt["f"][:, k, :], in_=w_v["f"][:, k, :])
            nc.sync.dma_start(out=wt["i"][:], in_=w_v["i"][:])
            nc.sync.dma_start(out=wt["h"][:], in_=w_v["h"][:])

            uf = [work.tile([P, ne], f32, tag="uf", name="uf_t") for _ in range(IT)]
            ui = [work.tile([P, ne], f32, tag="ui", name="ui_t") for _ in range(IT)]
            sg = [work.tile([P, ne], f32, tag="sg", name="sg_t") for _ in range(IT)]
            gt = [work.tile([P, ne], f32, tag="gt", name="gt_t") for _ in range(IT)]

            # f gate: k-outer across all 8 PSUM banks (2 halves x 4 h-tiles)
            # so each k-step consumes one (x[k], wf[k]) DMA pair.
            psf = [[pspool.tile([P, NT], f32, tag="ps", name="psf_t")
                    for _ in range(IT)] for _ in range(nhalf)]
            for k in range(KT):
                for half in range(nhalf):
                    for i in range(IT):
                        nc.tensor.matmul(
                            psf[half][i][:], lhsT=wt["f"][:, k, hsls[i]],
                            rhs=xj0[k][:, esls[half]],
                            start=(k == 0), stop=(k == KT - 1))
            for half in range(nhalf):
                for i in range(IT):
                    nc.scalar.activation(uf[i][:, esls[half]], psf[half][i][:],
                                         AF.Sigmoid, bias=bias_ap(0, i), scale=1.0)

            # i gate: bank-outer k-inner, chasing the freed f banks.
            for half in range(nhalf):
                for i in range(IT):
                    pst = pspool.tile([P, NT], f32, tag="ps", name="ps_t")
                    for k in range(KT):
                        nc.tensor.matmul(
                            pst[:], lhsT=wt["i"][:, k, hsls[i]],
                            rhs=xj0[k][:, esls[half]],
                            start=(k == 0), stop=(k == KT - 1))
                    nc.scalar.activation(ui[i][:, esls[half]], pst[:],
                                         AF.Sigmoid, bias=bias_ap(1, i), scale=1.0)
            for i in range(IT):
                fgate(uf[i], ui[i], ne)

            # h gate: bank-outer k-inner.
            for half in range(nhalf):
                for i in range(IT):
                    pst = pspool.tile([P, NT], f32, tag="ps", name="ps_t")
                    for k in range(KT):
                        nc.tensor.matmul(
                            pst[:], lhsT=wt["h"][:, k, hsls[i]],
                            rhs=xj0[k][:, esls[half]],
                            start=(k == 0), stop=(k == KT - 1))
                    nc.scalar.activation(sg[i][:, esls[half]], pst[:],
                                         AF.Sigmoid, bias=bias_ap(2, i), scale=1.0)
                    gmax(gt[i][:, esls[half]], pst[:], sg[i][:, esls[half]],
                         bias_ap(3, i))
            for i in range(IT):
                tail(i, ui[i], gt[i], 0, t0, ne)

            # ---- J1+: h-tile-major units ----
            for J, (t0, ne) in enumerate(CHUNKS[1:], start=1):
                halves = [(h0, min(NT, ne - h0)) for h0 in range(0, ne, NT)]
                xcs = []
                for h0, nt in halves:
                    xc = xpool.tile([P, KT, nt], MM_DT, tag="xh", name="xh_t")
                    nc.sync.dma_start(out=xc[:], in_=xT_v[:, :, t0 + h0:t0 + h0 + nt])
                    xcs.append(xc)
                for i in range(IT):
                    hsl = hsls[i]
                    ufu = work.tile([P, ne], f32, tag="uf", name="uf_t")
                    uiu = work.tile([P, ne], f32, tag="ui", name="ui_t")
                    sgu = work.tile([P, ne], f32, tag="sg", name="sg_t")
                    gtu = work.tile([P, ne], f32, tag="gt", name="gt_t")
                    for hi, (h0, nt) in enumerate(halves):
                        esl = slice(h0, h0 + nt)
                        for gate, dst, bk in (("f", ufu, 0), ("i", uiu, 1)):
                            pst = pspool.tile([P, NT], f32, tag="ps", name="ps_t")
                            for k in range(KT):
                                nc.tensor.matmul(
                                    pst[:, :nt], lhsT=wt[gate][:, k, hsl],
                                    rhs=xcs[hi][:, k, :],
                                    start=(k == 0), stop=(k == KT - 1))
                            nc.scalar.activation(dst[:, esl], pst[:, :nt],
                                                 AF.Sigmoid, bias=bias_ap(bk, i),
                                                 scale=1.0)
                    fgate(ufu, uiu, ne)
                    for hi, (h0, nt) in enumerate(halves):
                        esl = slice(h0, h0 + nt)
                        pst = pspool.tile([P, NT], f32, tag="ps", name="ps_t")
                        for k in range(KT):
                            nc.tensor.matmul(
                                pst[:, :nt], lhsT=wt["h"][:, k, hsl],
                                rhs=xcs[hi][:, k, :],
                                start=(k == 0), stop=(k == KT - 1))
                        nc.scalar.activation(sgu[:, esl], pst[:, :nt],
                                             AF.Sigmoid, bias=bias_ap(2, i),
                                             scale=1.0)
                        gmax(gtu[:, esl], pst[:, :nt], sgu[:, esl],
                             bias_ap(3, i))
                    tail(i, uiu, gtu, J, t0, ne)

    nc.compile()
    return nc


def _in_maps(x, W_f, b_f, W_i, b_i, W_h, b_h):
    x = np.asarray(x, np.float32)
    wT = {g: np.ascontiguousarray(np.asarray(w, np.float32).T).astype(np.float16)
          for g, w in (("f", W_f), ("i", W_i), ("h", W_h))}
    bs = {g: np.asarray(b, np.float32) for g, b in (("f", b_f), ("i", b_i), ("h", b_h))}

    maps = []
    for c in range(N_CORES):
        b, hh = divmod(c, 2)
        hsl = slice(hh * HSH, (hh + 1) * HSH)
        bias_pack = np.concatenate([
            bs["f"][hsl].reshape(IT, P).T,
            bs["i"][hsl].reshape(IT, P).T,
            bs["h"][hsl].reshape(IT, P).T,
            (bs["h"][hsl] + 0.5).reshape(IT, P).T,
        ], axis=1)
        maps.append({
            "xT": np.ascontiguousarray(x[b].T).astype(np.float16),
            "wf": np.ascontiguousarray(wT["f"][:, hsl]),
            "wi": np.ascontiguousarray(wT["i"][:, hsl]),
            "wh": np.ascontiguousarray(wT["h"][:, hsl]),
            "biases": np.ascontiguousarray(bias_pack, dtype=np.float32),
        })
    return maps


def kernel(x, W_f, b_f, W_i, b_i, W_h, b_h):
    global _COMPILED
    if _COMPILED is None:
        _COMPILED = _build()
    nc = _COMPILED

    res = run_bass_kernel_spmd(
        nc, _in_maps(x, W_f, b_f, W_i, b_i, W_h, b_h), list(range(N_CORES)))

    full = np.empty((B, T, DH), np.float32)
    for c in range(N_CORES):
        b, hh = divmod(c, 2)
        full[b, :, hh * HSH:(hh + 1) * HSH] = res.results[c]["out"].T
    return full
